# revision 3
# baseline (speedup 1.0000x reference)
"""Trainium2 Bass kernel v2 for the 2-layer edge-weighted GCN.

Math (matches reference.py):
    w_e   = softplus(edge_feats @ We + be)            per edge
    deg_d = sum_{e: dst=d} w_e + 1                    (self loop w=1)
    dinv  = 1/sqrt(deg)
    x1    = tanh(D^-1/2 (A+I) D^-1/2 X @ W1 + b1)
    out   = D^-1/2 (A+I) D^-1/2 x1s @ W2 + b2 + X @ Ws + bs

Distribution: equal node ranges of 6250 per core (edges partitioned by dst).
Groups = uniform 128-node windows (NG=49, identical structure on every core
=> valid SPMD).  Edge tiles of 128 are split per group into "low" (src row
< 32768) and "high" tiles so the int16-indexed batched dma_gather can
address the full 50000-row tables via a base-offset view.

Heavy path all bf16: one-hot scatter tiles (DVE batched is_equal + weight
mult with stride-0 broadcast APs), PE scatter-matmuls accumulating in fp32
PSUM, batched SWDGE dma_gather of 256B/512B rows, bf16 AllGathers.
"""

import os
import sys

import numpy as np
import ml_dtypes

for _p in ("/opt/trn_rl_repo",):
    if _p not in sys.path and os.path.isdir(_p):
        sys.path.insert(0, _p)

# ---------------- problem constants (hardcoded per spec) ----------------
N_NODES = 50000
N_EDGES = 800000
D_EDGE = 8
D_IN = 128
D_HID = 256
D_OUT = 256
NCORES = 8
P = 128
NPC = N_NODES // NCORES          # 6250 nodes per core
NG = -(-NPC // P)                # 49 groups of <=128 nodes
BASE16 = 32768                   # int16 split threshold for dma_gather

BF16 = ml_dtypes.bfloat16


# ======================================================================
# Host-side preprocessing
# ======================================================================

def _preprocess(edge_index):
    src = np.asarray(edge_index[0]).astype(np.int64)
    dst = np.asarray(edge_index[1]).astype(np.int64)

    order = np.argsort(dst, kind="stable")
    src_s = src[order]
    dst_s = dst[order]
    cnt = np.bincount(dst_s, minlength=N_NODES)
    cum = np.concatenate([[0], np.cumsum(cnt)])

    TG = np.zeros((NCORES, NG), int)
    for k in range(NCORES):
        lo = k * NPC
        for g in range(NG):
            a = lo + g * P
            b = min(a + P, lo + NPC)
            TG[k, g] = -(-int(cum[b] - cum[a]) // P)
    TGm = np.maximum(TG.max(axis=0), 1)
    cg = np.concatenate([[0], np.cumsum(TGm)])
    T2 = int(cg[-1])

    # local-first eligibility: group g gets a pre-AG local tile only if every
    # core has >= P local-src edges in that group
    LC = np.ones(NG, dtype=int)
    for k in range(NCORES):
        lo = k * NPC
        for g in range(NG):
            a = lo + g * P
            b = min(a + P, lo + NPC)
            sg = src_s[int(cum[a]):int(cum[b])]
            if int(((sg >= lo) & (sg < lo + NPC)).sum()) < P:
                LC[g] = 0

    per_core = []
    for k in range(NCORES):
        lo = k * NPC
        dstoff = np.zeros((P, T2), dtype=np.float32)
        idxg = np.zeros((P, T2), dtype=np.int32)
        eperm = np.full((P, T2), -1, dtype=np.int64)
        for g in range(NG):
            a = lo + g * P
            b = min(a + P, lo + NPC)
            ea, eb = int(cum[a]), int(cum[b])
            n = eb - ea
            if n == 0:
                continue
            sg = src_s[ea:eb]
            # local-first ordering: first P local-src edges feed the
            # pre-AG gather from the local table (tile 0 of the group)
            if LC[g]:
                locm = (sg >= lo) & (sg < lo + NPC)
                li = np.nonzero(locm)[0]
                ri = np.concatenate([li[P:], np.nonzero(~locm)[0]])
                perm = np.concatenate([li[:P], ri])
            else:
                perm = np.arange(n)
            sgp = sg[perm]
            i = np.arange(n)
            cc = int(cg[g]) + i // P
            pp = i % P
            iv = sgp.astype(np.int32).copy()
            if LC[g]:
                iv[:P] -= lo  # tile 0: local-table rows
            idxg[pp, cc] = iv
            dstoff[pp, cc] = (dst_s[ea:eb][perm] - a).astype(np.float32)
            eperm[pp, cc] = ea + perm
        per_core.append({"dstoff": dstoff, "idxg": idxg, "eperm": eperm})

    return {
        "order": order,
        "T2": T2,
        "TGm": TGm.astype(int).tolist(),
        "LC": LC.astype(int).tolist(),
        "cg": cg.astype(int).tolist(),
        "per_core": per_core,
    }


def _host_inputs(prep, edge_feats, node_feats, We):
    T2 = prep["T2"]
    ef_s = np.asarray(edge_feats, dtype=np.float32)[prep["order"]]
    X = np.ascontiguousarray(np.asarray(node_feats, dtype=np.float32))
    We = np.asarray(We, np.float32).reshape(D_EDGE)

    # padding edge-features that make softplus(ef@We+be) == 0
    pad_ef = (-60.0 / float(We @ We)) * We

    maps = []
    for k, pc in enumerate(prep["per_core"]):
        efq = np.empty((P, D_EDGE * T2), dtype=np.float32)
        for kf in range(D_EDGE):
            efq[:, kf * T2:(kf + 1) * T2] = pad_ef[kf]
        valid = pc["eperm"] >= 0
        pv, cv = np.nonzero(valid)
        eids = pc["eperm"][pv, cv]
        for kf in range(D_EDGE):
            efq[pv, kf * T2 + cv] = ef_s[eids, kf]

        lo = k * NPC
        xwin = np.zeros((NG * P, D_IN), dtype=np.float32)
        xwin[:NPC] = X[lo:lo + NPC]
        xwT = np.ascontiguousarray(xwin.T).astype(BF16)

        # unweighted one-hot tiles for the degree pass (DMA-loaded instead of
        # DVE-built: removes the serial is_equal block from the critical path)
        ptu = (np.arange(P, dtype=np.float32)[None, None, :]
               == pc["dstoff"][:, :, None]).astype(BF16)
        ptu[pc["eperm"] < 0] = 0
        maps.append(
            {
                "dstoffb": pc["dstoff"].astype(BF16),
                "idxg": pc["idxg"],
                "ptu": np.ascontiguousarray(ptu.reshape(P, -1)),
                "efq": efq,
                "xwin": xwin,
                "xwinTb": xwT,
            }
        )
    return maps


# ======================================================================
# Bass program
# ======================================================================

def _build_program(T2, TGm_l, cg, LC, debug=False):
    import concourse.bacc as bacc
    import concourse.bass as bass
    import concourse.mybir as mybir
    from concourse.masks import make_identity
    from concourse.tile import TileContext

    f32 = mybir.dt.float32
    bf16 = mybir.dt.bfloat16
    i32 = mybir.dt.int32
    i16 = mybir.dt.int16
    AF = mybir.ActivationFunctionType
    OP = mybir.AluOpType

    TGm = list(TGm_l)
    TGmax = max(TGm)
    NQGATHER = 4
    NROWS = NG * P  # 6272 padded rows per core

    nc = bacc.Bacc(trn_type="TRN2", num_devices=NCORES, num_swdge_queues=4)

    # ---- I/O ----
    dstoffb_t = nc.dram_tensor("dstoffb", [P, T2], bf16, kind="ExternalInput")
    ptu_t = nc.dram_tensor("ptu", [P, T2 * P], bf16, kind="ExternalInput")
    idxg_t = nc.dram_tensor("idxg", [P, T2], i32, kind="ExternalInput")
    efq_t = nc.dram_tensor("efq", [P, D_EDGE * T2], f32, kind="ExternalInput")
    xwin_t = nc.dram_tensor("xwin", [NROWS, D_IN], f32, kind="ExternalInput")
    xwinTb_t = nc.dram_tensor("xwinTb", [D_IN, NROWS], bf16, kind="ExternalInput")
    We_t = nc.dram_tensor("We_r", [1, D_EDGE], f32, kind="ExternalInput")
    be_t = nc.dram_tensor("be_r", [1, 1], f32, kind="ExternalInput")
    W1b_t = nc.dram_tensor("W1b", [D_IN, D_HID], bf16, kind="ExternalInput")
    W2b_t = nc.dram_tensor("W2b", [D_HID, D_OUT], bf16, kind="ExternalInput")
    Wsb_t = nc.dram_tensor("Wsb", [D_IN, D_OUT], bf16, kind="ExternalInput")
    b1bc_t = nc.dram_tensor("b1bc", [P, D_HID], bf16, kind="ExternalInput")
    b2bc_t = nc.dram_tensor("b2bc", [P, D_OUT], bf16, kind="ExternalInput")

    g1_loc = nc.dram_tensor("g1_loc", [NPC, D_IN], bf16, kind="Internal")
    g1_full = nc.dram_tensor(
        "g1_full", [N_NODES, D_IN], bf16, kind="Internal", addr_space="Shared"
    )
    x1s_loc = nc.dram_tensor("x1s_loc", [NPC, D_HID], bf16, kind="Internal")
    x1s_full = nc.dram_tensor(
        "x1s_full", [N_NODES, D_HID], bf16, kind="Internal", addr_space="Shared"
    )
    out_loc = nc.dram_tensor("out_loc", [NROWS, D_OUT], f32, kind="ExternalOutput")

    rg = [list(range(NCORES))]

    with TileContext(nc) as tc:
        with (
            tc.tile_pool(name="const", bufs=1) as cpool,
            tc.tile_pool(name="big", bufs=1) as bpool,
            tc.tile_pool(name="efb", bufs=2) as efpool,
            tc.tile_pool(name="gat1", bufs=3) as g1pool,
            tc.tile_pool(name="gat2", bufs=3) as g2pool,
            tc.tile_pool(name="ptp", bufs=4) as ppool,
            tc.tile_pool(name="eps", bufs=4) as epool,
            tc.tile_pool(name="psum", bufs=2, space="PSUM") as pspool,
            tc.tile_pool(name="psmm", bufs=2, space="PSUM") as pmpool,
        ):
            # ---------------- constants / weights ----------------
            iota_i = cpool.tile([P, P], dtype=i32)
            nc.gpsimd.iota(iota_i[:], pattern=[[1, P]], base=0, channel_multiplier=0)
            iota_b = cpool.tile([P, P], dtype=bf16)
            nc.vector.tensor_copy(iota_b[:], iota_i[:])

            ident = cpool.tile([P, P], dtype=bf16)
            make_identity(nc, ident[:])

            oinv = cpool.tile([P, P], dtype=bf16)
            nc.gpsimd.memset(oinv[:], 1.0 / 128.0)

            W1s = cpool.tile([D_IN, D_HID], dtype=bf16)
            nc.sync.dma_start(out=W1s[:], in_=W1b_t[:, :])
            W2a = cpool.tile([P, D_OUT], dtype=bf16)
            nc.sync.dma_start(out=W2a[:], in_=W2b_t[0:P, :])
            W2c = cpool.tile([P, D_OUT], dtype=bf16)
            nc.sync.dma_start(out=W2c[:], in_=W2b_t[P:2 * P, :])
            Wss = cpool.tile([D_IN, D_OUT], dtype=bf16)
            nc.sync.dma_start(out=Wss[:], in_=Wsb_t[:, :])
            b1bc = cpool.tile([P, D_HID], dtype=bf16)
            nc.sync.dma_start(out=b1bc[:], in_=b1bc_t[:, :])
            b2bc = cpool.tile([P, D_OUT], dtype=bf16)
            nc.sync.dma_start(out=b2bc[:], in_=b2bc_t[:, :])

            we_ld = cpool.tile([1, D_EDGE], dtype=f32)
            nc.sync.dma_start(out=we_ld[:], in_=We_t[:, :])
            WeB = cpool.tile([P, D_EDGE], dtype=f32)
            nc.gpsimd.partition_broadcast(WeB[:], we_ld[:1, :])
            be_ld = cpool.tile([1, 1], dtype=f32)
            nc.sync.dma_start(out=be_ld[:], in_=be_t[:, :])
            beB = cpool.tile([P, 1], dtype=f32)
            nc.gpsimd.partition_broadcast(beB[:], be_ld[:1, :])

            # ---------------- resident edge data ----------------
            dstb = bpool.tile([P, T2], dtype=bf16)
            nc.sync.dma_start(out=dstb[:], in_=dstoffb_t[:, :])
            idxT = bpool.tile([P, T2], dtype=i32)
            nc.sync.dma_start(out=idxT[:], in_=idxg_t[:, :])
            xwTb = bpool.tile([D_IN, NROWS], dtype=bf16)
            nc.sync.dma_start(out=xwTb[:], in_=xwinTb_t[:, :])

            wT = bpool.tile([P, T2], dtype=f32)
            wTb = bpool.tile([P, T2], dtype=bf16)
            dinvT = bpool.tile([P, NG], dtype=f32)
            selfT = bpool.tile([P, NG * D_IN], dtype=bf16)
            x1sT = bpool.tile([P, NG * D_HID], dtype=bf16)

            # ---------------- stage A: edge weights ----------------
            tmpA = bpool.tile([P, T2], dtype=f32)
            tmpB = bpool.tile([P, T2], dtype=f32)
            for kf in range(D_EDGE):
                efb = efpool.tile([P, T2], dtype=f32, tag="efb")
                nc.sync.dma_start(out=efb[:], in_=efq_t[:, kf * T2:(kf + 1) * T2])
                if kf == 0:
                    nc.vector.tensor_scalar(
                        out=wT[:], in0=efb[:], scalar1=WeB[:, 0:1], scalar2=None,
                        op0=OP.mult,
                    )
                else:
                    nc.vector.scalar_tensor_tensor(
                        out=wT[:], in0=efb[:], scalar=WeB[:, kf:kf + 1],
                        in1=wT[:], op0=OP.mult, op1=OP.add,
                    )
            nc.vector.tensor_scalar(
                out=wT[:], in0=wT[:], scalar1=beB[:, 0:1], scalar2=None, op0=OP.add
            )
            # softplus(x) = relu(x) + log(1 + exp(-|x|))
            nc.scalar.activation(out=tmpA[:], in_=wT[:], func=AF.Abs)
            nc.scalar.activation(out=tmpA[:], in_=tmpA[:], func=AF.Exp, scale=-1.0)
            nc.scalar.activation(out=tmpA[:], in_=tmpA[:], func=AF.Ln, bias=1.0)
            nc.scalar.activation(out=tmpB[:], in_=wT[:], func=AF.Relu)
            nc.vector.tensor_tensor(out=wT[:], in0=tmpA[:], in1=tmpB[:], op=OP.add)
            nc.vector.tensor_copy(wTb[:], wT[:])

            # ---------------- stage B: degree -> dinv, g1, selfT ----------
            for g in range(NG):
                a, tg = cg[g], TGm[g]
                ptb = ppool.tile([P, TGmax * P], dtype=bf16, tag="pt")
                nc.sync.dma_start(
                    out=ptb[:, :tg * P], in_=ptu_t[:, a * P:(a + tg) * P]
                )
                degp = pspool.tile([P, 1], dtype=f32, tag="acc")
                for t in range(tg):
                    nc.tensor.matmul(
                        degp[:], lhsT=ptb[:, t * P:(t + 1) * P],
                        rhs=wTb[:, a + t:a + t + 1],
                        start=(t == 0), stop=(t == tg - 1),
                    )
                # dinv = 1/sqrt(deg + 1)
                sq = epool.tile([P, 1], dtype=f32, tag="sq")
                nc.scalar.activation(out=sq[:], in_=degp[:], func=AF.Sqrt, bias=1.0)
                nc.vector.reciprocal(dinvT[:, g:g + 1], sq[:])
                # g1 = dinv * X, selfT = dinv^2 * X
                xwb = epool.tile([P, D_IN], dtype=f32, tag="xwb")
                nc.sync.dma_start(out=xwb[:], in_=xwin_t[g * P:(g + 1) * P, :])
                g1b = epool.tile([P, D_IN], dtype=bf16, tag="g1b")
                nc.vector.tensor_scalar(
                    out=g1b[:], in0=xwb[:], scalar1=dinvT[:, g:g + 1],
                    scalar2=None, op0=OP.mult,
                )
                nc.vector.tensor_scalar(
                    out=selfT[:, g * D_IN:(g + 1) * D_IN], in0=g1b[:],
                    scalar1=dinvT[:, g:g + 1], scalar2=None, op0=OP.mult,
                )
                rows = min(NPC - g * P, P)
                nc.sync.dma_start(
                    out=g1_loc[g * P:g * P + rows, :], in_=g1b[:rows, :]
                )

            # ---------------- allgather g1 ----------------
            nc.gpsimd.collective_compute(
                "AllGather", OP.bypass, rg, ins=[g1_loc[:, :]], outs=[g1_full[:, :]]
            )

            # local-src gathers (tile 0 of each group) read g1_loc and run
            # during the AllGather
            locg1 = bpool.tile([P, NG, D_IN], dtype=bf16)
            for g in range(NG):
                if LC[g]:
                    nc.gpsimd.indirect_dma_start(
                        out=locg1[:, g, :], out_offset=None, in_=g1_loc[:, :],
                        in_offset=bass.IndirectOffsetOnAxis(
                            ap=idxT[:, cg[g]:cg[g] + 1], axis=0),
                    )

            # ---------------- layer 1 ----------------
            for g in range(NG):
                a, tg = cg[g], TGm[g]
                gat = g1pool.tile([P, TGmax, D_IN], dtype=bf16, tag="gat1")
                for t in range(1 if LC[g] else 0, tg):
                    inst = nc.gpsimd.indirect_dma_start(
                        out=gat[:, t, :], out_offset=None, in_=g1_full[:, :],
                        in_offset=bass.IndirectOffsetOnAxis(
                            ap=idxT[:, a + t:a + t + 1], axis=0),
                    )
                    q = (a + t) % NQGATHER
                    if q:
                        inst.queue = f"qPoolDynamic{q}"

                ptb = ppool.tile([P, TGmax * P], dtype=bf16, tag="pt")
                nc.vector.tensor_tensor(
                    out=ptb[:, :tg * P],
                    in0=iota_b[:].unsqueeze(1).broadcast_to([P, tg, P]),
                    in1=dstb[:, a:a + tg].unsqueeze(2).broadcast_to([P, tg, P]),
                    op=OP.is_equal,
                )
                nc.vector.tensor_tensor(
                    out=ptb[:, :tg * P], in0=ptb[:, :tg * P],
                    in1=wTb[:, a:a + tg].unsqueeze(2).broadcast_to([P, tg, P]),
                    op=OP.mult,
                )
                aggp = pspool.tile([P, D_IN], dtype=f32, tag="acc")
                for t in range(tg):
                    nc.tensor.matmul(
                        aggp[:], lhsT=ptb[:, t * P:(t + 1) * P],
                        rhs=(locg1[:, g, :] if (t == 0 and LC[g]) else gat[:, t, :]),
                        start=(t == 0), stop=(t == tg - 1),
                    )
                # ax = dinv*agg + selfT
                axb = epool.tile([P, D_IN], dtype=bf16, tag="axb")
                nc.vector.scalar_tensor_tensor(
                    out=axb[:], in0=aggp[:], scalar=dinvT[:, g:g + 1],
                    in1=selfT[:, g * D_IN:(g + 1) * D_IN],
                    op0=OP.mult, op1=OP.add,
                )
                trp = pmpool.tile([P, P], dtype=bf16, tag="trp")
                nc.tensor.transpose(out=trp[:], in_=axb[:], identity=ident[:])
                axT = epool.tile([P, P], dtype=bf16, tag="axT")
                nc.vector.tensor_copy(axT[:], trp[:])
                o1p = pmpool.tile([P, D_HID], dtype=f32, tag="mm")
                nc.tensor.matmul(o1p[:], lhsT=axT[:], rhs=W1s[:], start=True,
                                 stop=False)
                nc.tensor.matmul(o1p[:], lhsT=oinv[:], rhs=b1bc[:], start=False,
                                 stop=True)
                x1t = epool.tile([P, D_HID], dtype=bf16, tag="x1t")
                nc.scalar.activation(out=x1t[:], in_=o1p[:], func=AF.Tanh)
                nc.vector.tensor_scalar(
                    out=x1sT[:, g * D_HID:(g + 1) * D_HID], in0=x1t[:],
                    scalar1=dinvT[:, g:g + 1], scalar2=None, op0=OP.mult,
                )
                rows = min(NPC - g * P, P)
                nc.sync.dma_start(
                    out=x1s_loc[g * P:g * P + rows, :],
                    in_=x1sT[:rows, g * D_HID:(g + 1) * D_HID],
                )

            # ---------------- allgather x1s ----------------
            nc.gpsimd.collective_compute(
                "AllGather", OP.bypass, rg, ins=[x1s_loc[:, :]],
                outs=[x1s_full[:, :]],
            )

            # local-src gathers overlap the AllGather
            locx1 = bpool.tile([P, NG, D_HID], dtype=bf16)
            for g in range(NG):
                if LC[g]:
                    nc.gpsimd.indirect_dma_start(
                        out=locx1[:, g, :], out_offset=None, in_=x1s_loc[:, :],
                        in_offset=bass.IndirectOffsetOnAxis(
                            ap=idxT[:, cg[g]:cg[g] + 1], axis=0),
                    )

            # ---------------- layer 2 + skip ----------------
            for g in range(NG):
                a, tg = cg[g], TGm[g]
                gat2 = g2pool.tile([P, TGmax, D_HID], dtype=bf16, tag="gat2")
                for t in range(1 if LC[g] else 0, tg):
                    inst = nc.gpsimd.indirect_dma_start(
                        out=gat2[:, t, :], out_offset=None, in_=x1s_full[:, :],
                        in_offset=bass.IndirectOffsetOnAxis(
                            ap=idxT[:, a + t:a + t + 1], axis=0),
                    )
                    q = (a + t) % NQGATHER
                    if q:
                        inst.queue = f"qPoolDynamic{q}"

                ptb = ppool.tile([P, TGmax * P], dtype=bf16, tag="pt")
                nc.vector.tensor_tensor(
                    out=ptb[:, :tg * P],
                    in0=iota_b[:].unsqueeze(1).broadcast_to([P, tg, P]),
                    in1=dstb[:, a:a + tg].unsqueeze(2).broadcast_to([P, tg, P]),
                    op=OP.is_equal,
                )
                nc.vector.tensor_tensor(
                    out=ptb[:, :tg * P], in0=ptb[:, :tg * P],
                    in1=wTb[:, a:a + tg].unsqueeze(2).broadcast_to([P, tg, P]),
                    op=OP.mult,
                )
                agg2 = pspool.tile([P, D_HID], dtype=f32, tag="acc")
                for t in range(tg):
                    nc.tensor.matmul(
                        agg2[:], lhsT=ptb[:, t * P:(t + 1) * P],
                        rhs=(locx1[:, g, :] if (t == 0 and LC[g]) else gat2[:, t, :]),
                        start=(t == 0), stop=(t == tg - 1),
                    )
                # ax2 = dinv * (agg2 + x1s_own)
                t3 = epool.tile([P, D_HID], dtype=bf16, tag="t3")
                nc.vector.tensor_tensor(
                    out=t3[:], in0=agg2[:],
                    in1=x1sT[:, g * D_HID:(g + 1) * D_HID], op=OP.add,
                )
                ax2 = epool.tile([P, D_HID], dtype=bf16, tag="ax2")
                nc.vector.tensor_scalar(
                    out=ax2[:], in0=t3[:], scalar1=dinvT[:, g:g + 1],
                    scalar2=None, op0=OP.mult,
                )
                o2p = pmpool.tile([P, D_OUT], dtype=f32, tag="mm")
                for h in range(2):
                    trp2 = pmpool.tile([P, P], dtype=bf16, tag="trp")
                    nc.tensor.transpose(
                        out=trp2[:], in_=ax2[:, h * P:(h + 1) * P],
                        identity=ident[:],
                    )
                    ax2T = epool.tile([P, P], dtype=bf16, tag=f"ax2T{h}")
                    nc.vector.tensor_copy(ax2T[:], trp2[:])
                    nc.tensor.matmul(
                        o2p[:], lhsT=ax2T[:], rhs=(W2a[:] if h == 0 else W2c[:]),
                        start=(h == 0), stop=False,
                    )
                nc.tensor.matmul(
                    o2p[:], lhsT=xwTb[:, g * P:(g + 1) * P], rhs=Wss[:],
                    start=False, stop=False,
                )
                nc.tensor.matmul(
                    o2p[:], lhsT=oinv[:], rhs=b2bc[:], start=False, stop=True
                )
                ob = epool.tile([P, D_OUT], dtype=f32, tag="ob")
                nc.vector.tensor_copy(ob[:], o2p[:])
                nc.sync.dma_start(
                    out=out_loc[g * P:(g + 1) * P, :], in_=ob[:]
                )

    nc.compile()
    return nc


# ======================================================================
# Driver
# ======================================================================

_CACHE = {}


def _get_program(T2, TGm, cg, LC):
    key = ("prog", T2, tuple(TGm), tuple(LC))
    if key not in _CACHE:
        _CACHE[key] = _build_program(T2, TGm, cg, LC)
    return _CACHE[key]


def _run(inputs, trace=False):
    from concourse.bass_utils import run_bass_kernel_spmd

    edge_index = np.asarray(inputs["edge_index"])
    ei_key = hash(edge_index.tobytes())
    pkey = ("prep", ei_key)
    if pkey not in _CACHE:
        _CACHE[pkey] = _preprocess(edge_index)
    prep = _CACHE[pkey]

    nc = _get_program(prep["T2"], prep["TGm"], prep["cg"], prep["LC"])
    maps = _host_inputs(prep, inputs["edge_feats"], inputs["node_feats"],
                        inputs["We"])

    b2s = (np.asarray(inputs["b2"], np.float32)
           + np.asarray(inputs["bs"], np.float32)).reshape(1, D_OUT)
    shared = {
        "We_r": np.asarray(inputs["We"], np.float32).reshape(1, D_EDGE),
        "be_r": np.asarray(inputs["be"], np.float32).reshape(1, 1),
        "W1b": np.asarray(inputs["W1"], np.float32).astype(BF16),
        "W2b": np.asarray(inputs["W2"], np.float32).astype(BF16),
        "Wsb": np.asarray(inputs["Ws"], np.float32).astype(BF16),
        "b1bc": np.tile(np.asarray(inputs["b1"], np.float32).reshape(1, D_HID),
                        (P, 1)).astype(BF16),
        "b2bc": np.tile(b2s, (P, 1)).astype(BF16),
    }
    in_maps = [{**m, **shared} for m in maps]

    res = run_bass_kernel_spmd(
        nc, in_maps, core_ids=list(range(NCORES)), trace=trace
    )

    out = np.empty((N_NODES, D_OUT), dtype=np.float32)
    for k in range(NCORES):
        out[k * NPC:(k + 1) * NPC] = res.results[k]["out_loc"][:NPC]
    return out, res


def kernel(**inputs):
    out, _ = _run(inputs, trace=False)
    return out


# revision 4
# speedup vs baseline: 1.0174x; 1.0174x over previous
"""Trainium2 Bass kernel v2 for the 2-layer edge-weighted GCN.

Math (matches reference.py):
    w_e   = softplus(edge_feats @ We + be)            per edge
    deg_d = sum_{e: dst=d} w_e + 1                    (self loop w=1)
    dinv  = 1/sqrt(deg)
    x1    = tanh(D^-1/2 (A+I) D^-1/2 X @ W1 + b1)
    out   = D^-1/2 (A+I) D^-1/2 x1s @ W2 + b2 + X @ Ws + bs

Distribution: equal node ranges of 6250 per core (edges partitioned by dst).
Groups = uniform 128-node windows (NG=49, identical structure on every core
=> valid SPMD).  Edge tiles of 128 are split per group into "low" (src row
< 32768) and "high" tiles so the int16-indexed batched dma_gather can
address the full 50000-row tables via a base-offset view.

Heavy path all bf16: one-hot scatter tiles (DVE batched is_equal + weight
mult with stride-0 broadcast APs), PE scatter-matmuls accumulating in fp32
PSUM, batched SWDGE dma_gather of 256B/512B rows, bf16 AllGathers.
"""

import os
import sys

import numpy as np
import ml_dtypes

for _p in ("/opt/trn_rl_repo",):
    if _p not in sys.path and os.path.isdir(_p):
        sys.path.insert(0, _p)

# ---------------- problem constants (hardcoded per spec) ----------------
N_NODES = 50000
N_EDGES = 800000
D_EDGE = 8
D_IN = 128
D_HID = 256
D_OUT = 256
NCORES = 8
P = 128
NPC = N_NODES // NCORES          # 6250 nodes per core
NG = -(-NPC // P)                # 49 groups of <=128 nodes
BASE16 = 32768                   # int16 split threshold for dma_gather

BF16 = ml_dtypes.bfloat16


# ======================================================================
# Host-side preprocessing
# ======================================================================

def _preprocess(edge_index):
    src = np.asarray(edge_index[0]).astype(np.int64)
    dst = np.asarray(edge_index[1]).astype(np.int64)

    order = np.argsort(dst, kind="stable")
    src_s = src[order]
    dst_s = dst[order]
    cnt = np.bincount(dst_s, minlength=N_NODES)
    cum = np.concatenate([[0], np.cumsum(cnt)])

    TG = np.zeros((NCORES, NG), int)
    for k in range(NCORES):
        lo = k * NPC
        for g in range(NG):
            a = lo + g * P
            b = min(a + P, lo + NPC)
            TG[k, g] = -(-int(cum[b] - cum[a]) // P)
    TGm = np.maximum(TG.max(axis=0), 1)
    cg = np.concatenate([[0], np.cumsum(TGm)])
    T2 = int(cg[-1])

    # local-first eligibility: group g gets a pre-AG local tile only if every
    # core has >= P local-src edges in that group
    LC = np.ones(NG, dtype=int)
    for k in range(NCORES):
        lo = k * NPC
        for g in range(NG):
            a = lo + g * P
            b = min(a + P, lo + NPC)
            sg = src_s[int(cum[a]):int(cum[b])]
            if int(((sg >= lo) & (sg < lo + NPC)).sum()) < P:
                LC[g] = 0

    per_core = []
    for k in range(NCORES):
        lo = k * NPC
        dstoff = np.zeros((P, T2), dtype=np.float32)
        idxg = np.zeros((P, T2), dtype=np.int32)
        eperm = np.full((P, T2), -1, dtype=np.int64)
        for g in range(NG):
            a = lo + g * P
            b = min(a + P, lo + NPC)
            ea, eb = int(cum[a]), int(cum[b])
            n = eb - ea
            if n == 0:
                continue
            sg = src_s[ea:eb]
            # local-first ordering: first P local-src edges feed the
            # pre-AG gather from the local table (tile 0 of the group)
            if LC[g]:
                locm = (sg >= lo) & (sg < lo + NPC)
                li = np.nonzero(locm)[0]
                ri = np.concatenate([li[P:], np.nonzero(~locm)[0]])
                perm = np.concatenate([li[:P], ri])
            else:
                perm = np.arange(n)
            sgp = sg[perm]
            i = np.arange(n)
            cc = int(cg[g]) + i // P
            pp = i % P
            iv = sgp.astype(np.int32).copy()
            if LC[g]:
                iv[:P] -= lo  # tile 0: local-table rows
            idxg[pp, cc] = iv
            dstoff[pp, cc] = (dst_s[ea:eb][perm] - a).astype(np.float32)
            eperm[pp, cc] = ea + perm
        per_core.append({"dstoff": dstoff, "idxg": idxg, "eperm": eperm})

    return {
        "order": order,
        "T2": T2,
        "TGm": TGm.astype(int).tolist(),
        "LC": LC.astype(int).tolist(),
        "cg": cg.astype(int).tolist(),
        "per_core": per_core,
    }


def _host_inputs(prep, edge_feats, node_feats, We):
    T2 = prep["T2"]
    ef_s = np.asarray(edge_feats, dtype=np.float32)[prep["order"]]
    X = np.ascontiguousarray(np.asarray(node_feats, dtype=np.float32))
    We = np.asarray(We, np.float32).reshape(D_EDGE)

    # padding edge-features that make softplus(ef@We+be) == 0
    pad_ef = (-60.0 / float(We @ We)) * We

    maps = []
    for k, pc in enumerate(prep["per_core"]):
        efq = np.empty((P, D_EDGE * T2), dtype=np.float32)
        for kf in range(D_EDGE):
            efq[:, kf * T2:(kf + 1) * T2] = pad_ef[kf]
        valid = pc["eperm"] >= 0
        pv, cv = np.nonzero(valid)
        eids = pc["eperm"][pv, cv]
        for kf in range(D_EDGE):
            efq[pv, kf * T2 + cv] = ef_s[eids, kf]

        lo = k * NPC
        xwin = np.zeros((NG * P, D_IN), dtype=np.float32)
        xwin[:NPC] = X[lo:lo + NPC]
        xwT = np.ascontiguousarray(xwin.T).astype(BF16)

        maps.append(
            {
                "dstoffb": pc["dstoff"].astype(BF16),
                "idxg": pc["idxg"],
                "efq": efq,
                "xwin": xwin,
                "xwinTb": xwT,
            }
        )
    return maps


# ======================================================================
# Bass program
# ======================================================================

def _build_program(T2, TGm_l, cg, LC, debug=False):
    import concourse.bacc as bacc
    import concourse.bass as bass
    import concourse.mybir as mybir
    from concourse.masks import make_identity
    from concourse.tile import TileContext

    f32 = mybir.dt.float32
    bf16 = mybir.dt.bfloat16
    i32 = mybir.dt.int32
    i16 = mybir.dt.int16
    AF = mybir.ActivationFunctionType
    OP = mybir.AluOpType

    TGm = list(TGm_l)
    TGmax = max(TGm)
    NQGATHER = 4
    NROWS = NG * P  # 6272 padded rows per core

    nc = bacc.Bacc(trn_type="TRN2", num_devices=NCORES, num_swdge_queues=4)

    # ---- I/O ----
    dstoffb_t = nc.dram_tensor("dstoffb", [P, T2], bf16, kind="ExternalInput")
    idxg_t = nc.dram_tensor("idxg", [P, T2], i32, kind="ExternalInput")
    efq_t = nc.dram_tensor("efq", [P, D_EDGE * T2], f32, kind="ExternalInput")
    xwin_t = nc.dram_tensor("xwin", [NROWS, D_IN], f32, kind="ExternalInput")
    xwinTb_t = nc.dram_tensor("xwinTb", [D_IN, NROWS], bf16, kind="ExternalInput")
    We_t = nc.dram_tensor("We_r", [1, D_EDGE], f32, kind="ExternalInput")
    be_t = nc.dram_tensor("be_r", [1, 1], f32, kind="ExternalInput")
    W1b_t = nc.dram_tensor("W1b", [D_IN, D_HID], bf16, kind="ExternalInput")
    W2b_t = nc.dram_tensor("W2b", [D_HID, D_OUT], bf16, kind="ExternalInput")
    Wsb_t = nc.dram_tensor("Wsb", [D_IN, D_OUT], bf16, kind="ExternalInput")
    b1bc_t = nc.dram_tensor("b1bc", [P, D_HID], bf16, kind="ExternalInput")
    b2bc_t = nc.dram_tensor("b2bc", [P, D_OUT], bf16, kind="ExternalInput")

    g1_loc = nc.dram_tensor("g1_loc", [NPC, D_IN], bf16, kind="Internal")
    g1_full = nc.dram_tensor(
        "g1_full", [N_NODES, D_IN], bf16, kind="Internal", addr_space="Shared"
    )
    x1s_loc = nc.dram_tensor("x1s_loc", [NPC, D_HID], bf16, kind="Internal")
    x1s_full = nc.dram_tensor(
        "x1s_full", [N_NODES, D_HID], bf16, kind="Internal", addr_space="Shared"
    )
    out_loc = nc.dram_tensor("out_loc", [NROWS, D_OUT], f32, kind="ExternalOutput")

    rg = [list(range(NCORES))]

    with TileContext(nc) as tc:
        with (
            tc.tile_pool(name="const", bufs=1) as cpool,
            tc.tile_pool(name="big", bufs=1) as bpool,
            tc.tile_pool(name="efb", bufs=2) as efpool,
            tc.tile_pool(name="gat1", bufs=3) as g1pool,
            tc.tile_pool(name="gat2", bufs=3) as g2pool,
            tc.tile_pool(name="ptp", bufs=4) as ppool,
            tc.tile_pool(name="eps", bufs=4) as epool,
            tc.tile_pool(name="psum", bufs=2, space="PSUM") as pspool,
            tc.tile_pool(name="psmm", bufs=2, space="PSUM") as pmpool,
        ):
            # ---------------- constants / weights ----------------
            iota_i = cpool.tile([P, P], dtype=i32)
            nc.gpsimd.iota(iota_i[:], pattern=[[1, P]], base=0, channel_multiplier=0)
            iota_b = cpool.tile([P, P], dtype=bf16)
            nc.vector.tensor_copy(iota_b[:], iota_i[:])

            ident = cpool.tile([P, P], dtype=bf16)
            make_identity(nc, ident[:])

            oinv = cpool.tile([P, P], dtype=bf16)
            nc.gpsimd.memset(oinv[:], 1.0 / 128.0)

            W1s = cpool.tile([D_IN, D_HID], dtype=bf16)
            nc.sync.dma_start(out=W1s[:], in_=W1b_t[:, :])
            W2a = cpool.tile([P, D_OUT], dtype=bf16)
            nc.sync.dma_start(out=W2a[:], in_=W2b_t[0:P, :])
            W2c = cpool.tile([P, D_OUT], dtype=bf16)
            nc.sync.dma_start(out=W2c[:], in_=W2b_t[P:2 * P, :])
            Wss = cpool.tile([D_IN, D_OUT], dtype=bf16)
            nc.sync.dma_start(out=Wss[:], in_=Wsb_t[:, :])
            b1bc = cpool.tile([P, D_HID], dtype=bf16)
            nc.sync.dma_start(out=b1bc[:], in_=b1bc_t[:, :])
            b2bc = cpool.tile([P, D_OUT], dtype=bf16)
            nc.sync.dma_start(out=b2bc[:], in_=b2bc_t[:, :])

            we_ld = cpool.tile([1, D_EDGE], dtype=f32)
            nc.sync.dma_start(out=we_ld[:], in_=We_t[:, :])
            WeB = cpool.tile([P, D_EDGE], dtype=f32)
            nc.gpsimd.partition_broadcast(WeB[:], we_ld[:1, :])
            be_ld = cpool.tile([1, 1], dtype=f32)
            nc.sync.dma_start(out=be_ld[:], in_=be_t[:, :])
            beB = cpool.tile([P, 1], dtype=f32)
            nc.gpsimd.partition_broadcast(beB[:], be_ld[:1, :])

            # ---------------- resident edge data ----------------
            dstb = bpool.tile([P, T2], dtype=bf16)
            nc.sync.dma_start(out=dstb[:], in_=dstoffb_t[:, :])
            idxT = bpool.tile([P, T2], dtype=i32)
            nc.sync.dma_start(out=idxT[:], in_=idxg_t[:, :])
            xwTb = bpool.tile([D_IN, NROWS], dtype=bf16)
            nc.sync.dma_start(out=xwTb[:], in_=xwinTb_t[:, :])

            wT = bpool.tile([P, T2], dtype=f32)
            wTb = bpool.tile([P, T2], dtype=bf16)
            dinvT = bpool.tile([P, NG], dtype=f32)
            selfT = bpool.tile([P, NG * D_IN], dtype=bf16)
            x1sT = bpool.tile([P, NG * D_HID], dtype=bf16)

            # ---------------- stage A: edge weights ----------------
            tmpA = bpool.tile([P, T2], dtype=f32)
            tmpB = bpool.tile([P, T2], dtype=f32)
            for kf in range(D_EDGE):
                efb = efpool.tile([P, T2], dtype=f32, tag="efb")
                nc.sync.dma_start(out=efb[:], in_=efq_t[:, kf * T2:(kf + 1) * T2])
                if kf == 0:
                    nc.vector.tensor_scalar(
                        out=wT[:], in0=efb[:], scalar1=WeB[:, 0:1], scalar2=None,
                        op0=OP.mult,
                    )
                else:
                    nc.vector.scalar_tensor_tensor(
                        out=wT[:], in0=efb[:], scalar=WeB[:, kf:kf + 1],
                        in1=wT[:], op0=OP.mult, op1=OP.add,
                    )
            nc.vector.tensor_scalar(
                out=wT[:], in0=wT[:], scalar1=beB[:, 0:1], scalar2=None, op0=OP.add
            )
            # softplus(x) = relu(x) + log(1 + exp(-|x|))
            nc.scalar.activation(out=tmpA[:], in_=wT[:], func=AF.Abs)
            nc.scalar.activation(out=tmpA[:], in_=tmpA[:], func=AF.Exp, scale=-1.0)
            nc.scalar.activation(out=tmpA[:], in_=tmpA[:], func=AF.Ln, bias=1.0)
            nc.scalar.activation(out=tmpB[:], in_=wT[:], func=AF.Relu)
            nc.vector.tensor_tensor(out=wT[:], in0=tmpA[:], in1=tmpB[:], op=OP.add)
            nc.vector.tensor_copy(wTb[:], wT[:])

            # ---------------- stage B: degree -> dinv, g1, selfT ----------
            for g in range(NG):
                a, tg = cg[g], TGm[g]
                ptb = ppool.tile([P, TGmax * P], dtype=bf16, tag="pt")
                nc.vector.tensor_tensor(
                    out=ptb[:, :tg * P],
                    in0=iota_b[:].unsqueeze(1).broadcast_to([P, tg, P]),
                    in1=dstb[:, a:a + tg].unsqueeze(2).broadcast_to([P, tg, P]),
                    op=OP.is_equal,
                )
                degp = pspool.tile([P, 1], dtype=f32, tag="acc")
                for t in range(tg):
                    nc.tensor.matmul(
                        degp[:], lhsT=ptb[:, t * P:(t + 1) * P],
                        rhs=wTb[:, a + t:a + t + 1],
                        start=(t == 0), stop=(t == tg - 1),
                    )
                # dinv = 1/sqrt(deg + 1)
                sq = epool.tile([P, 1], dtype=f32, tag="sq")
                nc.scalar.activation(out=sq[:], in_=degp[:], func=AF.Sqrt, bias=1.0)
                nc.vector.reciprocal(dinvT[:, g:g + 1], sq[:])
                # g1 = dinv * X, selfT = dinv^2 * X
                xwb = epool.tile([P, D_IN], dtype=f32, tag="xwb")
                nc.sync.dma_start(out=xwb[:], in_=xwin_t[g * P:(g + 1) * P, :])
                g1b = epool.tile([P, D_IN], dtype=bf16, tag="g1b")
                nc.vector.tensor_scalar(
                    out=g1b[:], in0=xwb[:], scalar1=dinvT[:, g:g + 1],
                    scalar2=None, op0=OP.mult,
                )
                nc.vector.tensor_scalar(
                    out=selfT[:, g * D_IN:(g + 1) * D_IN], in0=g1b[:],
                    scalar1=dinvT[:, g:g + 1], scalar2=None, op0=OP.mult,
                )
                rows = min(NPC - g * P, P)
                nc.sync.dma_start(
                    out=g1_loc[g * P:g * P + rows, :], in_=g1b[:rows, :]
                )

            # ---------------- allgather g1 ----------------
            nc.gpsimd.collective_compute(
                "AllGather", OP.bypass, rg, ins=[g1_loc[:, :]], outs=[g1_full[:, :]]
            )

            # local-src gathers (tile 0 of each group) read g1_loc and run
            # during the AllGather
            locg1 = bpool.tile([P, NG, D_IN], dtype=bf16)
            for g in range(NG):
                if LC[g]:
                    nc.gpsimd.indirect_dma_start(
                        out=locg1[:, g, :], out_offset=None, in_=g1_loc[:, :],
                        in_offset=bass.IndirectOffsetOnAxis(
                            ap=idxT[:, cg[g]:cg[g] + 1], axis=0),
                    )

            # ---------------- layer 1 ----------------
            for g in range(NG):
                a, tg = cg[g], TGm[g]
                gat = g1pool.tile([P, TGmax, D_IN], dtype=bf16, tag="gat1")
                for t in range(1 if LC[g] else 0, tg):
                    inst = nc.gpsimd.indirect_dma_start(
                        out=gat[:, t, :], out_offset=None, in_=g1_full[:, :],
                        in_offset=bass.IndirectOffsetOnAxis(
                            ap=idxT[:, a + t:a + t + 1], axis=0),
                    )
                    q = (a + t) % NQGATHER
                    if q:
                        inst.queue = f"qPoolDynamic{q}"

                ptb = ppool.tile([P, TGmax * P], dtype=bf16, tag="pt")
                nc.vector.tensor_tensor(
                    out=ptb[:, :tg * P],
                    in0=iota_b[:].unsqueeze(1).broadcast_to([P, tg, P]),
                    in1=dstb[:, a:a + tg].unsqueeze(2).broadcast_to([P, tg, P]),
                    op=OP.is_equal,
                )
                nc.vector.tensor_tensor(
                    out=ptb[:, :tg * P], in0=ptb[:, :tg * P],
                    in1=wTb[:, a:a + tg].unsqueeze(2).broadcast_to([P, tg, P]),
                    op=OP.mult,
                )
                aggp = pspool.tile([P, D_IN], dtype=f32, tag="acc")
                for t in range(tg):
                    nc.tensor.matmul(
                        aggp[:], lhsT=ptb[:, t * P:(t + 1) * P],
                        rhs=(locg1[:, g, :] if (t == 0 and LC[g]) else gat[:, t, :]),
                        start=(t == 0), stop=(t == tg - 1),
                    )
                # ax = dinv*agg + selfT
                axb = epool.tile([P, D_IN], dtype=bf16, tag="axb")
                nc.vector.scalar_tensor_tensor(
                    out=axb[:], in0=aggp[:], scalar=dinvT[:, g:g + 1],
                    in1=selfT[:, g * D_IN:(g + 1) * D_IN],
                    op0=OP.mult, op1=OP.add,
                )
                trp = pmpool.tile([P, P], dtype=bf16, tag="trp")
                nc.tensor.transpose(out=trp[:], in_=axb[:], identity=ident[:])
                axT = epool.tile([P, P], dtype=bf16, tag="axT")
                nc.vector.tensor_copy(axT[:], trp[:])
                o1p = pmpool.tile([P, D_HID], dtype=f32, tag="mm")
                nc.tensor.matmul(o1p[:], lhsT=axT[:], rhs=W1s[:], start=True,
                                 stop=False)
                nc.tensor.matmul(o1p[:], lhsT=oinv[:], rhs=b1bc[:], start=False,
                                 stop=True)
                x1t = epool.tile([P, D_HID], dtype=bf16, tag="x1t")
                nc.scalar.activation(out=x1t[:], in_=o1p[:], func=AF.Tanh)
                nc.vector.tensor_scalar(
                    out=x1sT[:, g * D_HID:(g + 1) * D_HID], in0=x1t[:],
                    scalar1=dinvT[:, g:g + 1], scalar2=None, op0=OP.mult,
                )
                rows = min(NPC - g * P, P)
                nc.sync.dma_start(
                    out=x1s_loc[g * P:g * P + rows, :],
                    in_=x1sT[:rows, g * D_HID:(g + 1) * D_HID],
                )

            # ---------------- allgather x1s ----------------
            nc.gpsimd.collective_compute(
                "AllGather", OP.bypass, rg, ins=[x1s_loc[:, :]],
                outs=[x1s_full[:, :]],
            )

            # local-src gathers overlap the AllGather
            locx1 = bpool.tile([P, NG, D_HID], dtype=bf16)
            for g in range(NG):
                if LC[g]:
                    nc.gpsimd.indirect_dma_start(
                        out=locx1[:, g, :], out_offset=None, in_=x1s_loc[:, :],
                        in_offset=bass.IndirectOffsetOnAxis(
                            ap=idxT[:, cg[g]:cg[g] + 1], axis=0),
                    )

            # ---------------- layer 2 + skip ----------------
            for g in range(NG):
                a, tg = cg[g], TGm[g]
                gat2 = g2pool.tile([P, TGmax, D_HID], dtype=bf16, tag="gat2")
                for t in range(1 if LC[g] else 0, tg):
                    inst = nc.gpsimd.indirect_dma_start(
                        out=gat2[:, t, :], out_offset=None, in_=x1s_full[:, :],
                        in_offset=bass.IndirectOffsetOnAxis(
                            ap=idxT[:, a + t:a + t + 1], axis=0),
                    )
                    q = (a + t) % NQGATHER
                    if q:
                        inst.queue = f"qPoolDynamic{q}"

                ptb = ppool.tile([P, TGmax * P], dtype=bf16, tag="pt")
                nc.vector.tensor_tensor(
                    out=ptb[:, :tg * P],
                    in0=iota_b[:].unsqueeze(1).broadcast_to([P, tg, P]),
                    in1=dstb[:, a:a + tg].unsqueeze(2).broadcast_to([P, tg, P]),
                    op=OP.is_equal,
                )
                nc.vector.tensor_tensor(
                    out=ptb[:, :tg * P], in0=ptb[:, :tg * P],
                    in1=wTb[:, a:a + tg].unsqueeze(2).broadcast_to([P, tg, P]),
                    op=OP.mult,
                )
                agg2 = pspool.tile([P, D_HID], dtype=f32, tag="acc")
                for t in range(tg):
                    nc.tensor.matmul(
                        agg2[:], lhsT=ptb[:, t * P:(t + 1) * P],
                        rhs=(locx1[:, g, :] if (t == 0 and LC[g]) else gat2[:, t, :]),
                        start=(t == 0), stop=(t == tg - 1),
                    )
                # ax2 = dinv * (agg2 + x1s_own)
                t3 = epool.tile([P, D_HID], dtype=bf16, tag="t3")
                nc.vector.tensor_tensor(
                    out=t3[:], in0=agg2[:],
                    in1=x1sT[:, g * D_HID:(g + 1) * D_HID], op=OP.add,
                )
                ax2 = epool.tile([P, D_HID], dtype=bf16, tag="ax2")
                nc.vector.tensor_scalar(
                    out=ax2[:], in0=t3[:], scalar1=dinvT[:, g:g + 1],
                    scalar2=None, op0=OP.mult,
                )
                o2p = pmpool.tile([P, D_OUT], dtype=f32, tag="mm")
                for h in range(2):
                    trp2 = pmpool.tile([P, P], dtype=bf16, tag="trp")
                    nc.tensor.transpose(
                        out=trp2[:], in_=ax2[:, h * P:(h + 1) * P],
                        identity=ident[:],
                    )
                    ax2T = epool.tile([P, P], dtype=bf16, tag=f"ax2T{h}")
                    nc.vector.tensor_copy(ax2T[:], trp2[:])
                    nc.tensor.matmul(
                        o2p[:], lhsT=ax2T[:], rhs=(W2a[:] if h == 0 else W2c[:]),
                        start=(h == 0), stop=False,
                    )
                nc.tensor.matmul(
                    o2p[:], lhsT=xwTb[:, g * P:(g + 1) * P], rhs=Wss[:],
                    start=False, stop=False,
                )
                nc.tensor.matmul(
                    o2p[:], lhsT=oinv[:], rhs=b2bc[:], start=False, stop=True
                )
                ob = epool.tile([P, D_OUT], dtype=f32, tag="ob")
                nc.vector.tensor_copy(ob[:], o2p[:])
                nc.sync.dma_start(
                    out=out_loc[g * P:(g + 1) * P, :], in_=ob[:]
                )

    nc.compile()
    return nc


# ======================================================================
# Driver
# ======================================================================

_CACHE = {}


def _get_program(T2, TGm, cg, LC):
    key = ("prog", T2, tuple(TGm), tuple(LC))
    if key not in _CACHE:
        _CACHE[key] = _build_program(T2, TGm, cg, LC)
    return _CACHE[key]


def _run(inputs, trace=False):
    from concourse.bass_utils import run_bass_kernel_spmd

    edge_index = np.asarray(inputs["edge_index"])
    ei_key = hash(edge_index.tobytes())
    pkey = ("prep", ei_key)
    if pkey not in _CACHE:
        _CACHE[pkey] = _preprocess(edge_index)
    prep = _CACHE[pkey]

    nc = _get_program(prep["T2"], prep["TGm"], prep["cg"], prep["LC"])
    maps = _host_inputs(prep, inputs["edge_feats"], inputs["node_feats"],
                        inputs["We"])

    b2s = (np.asarray(inputs["b2"], np.float32)
           + np.asarray(inputs["bs"], np.float32)).reshape(1, D_OUT)
    shared = {
        "We_r": np.asarray(inputs["We"], np.float32).reshape(1, D_EDGE),
        "be_r": np.asarray(inputs["be"], np.float32).reshape(1, 1),
        "W1b": np.asarray(inputs["W1"], np.float32).astype(BF16),
        "W2b": np.asarray(inputs["W2"], np.float32).astype(BF16),
        "Wsb": np.asarray(inputs["Ws"], np.float32).astype(BF16),
        "b1bc": np.tile(np.asarray(inputs["b1"], np.float32).reshape(1, D_HID),
                        (P, 1)).astype(BF16),
        "b2bc": np.tile(b2s, (P, 1)).astype(BF16),
    }
    in_maps = [{**m, **shared} for m in maps]

    res = run_bass_kernel_spmd(
        nc, in_maps, core_ids=list(range(NCORES)), trace=trace
    )

    out = np.empty((N_NODES, D_OUT), dtype=np.float32)
    for k in range(NCORES):
        out[k * NPC:(k + 1) * NPC] = res.results[k]["out_loc"][:NPC]
    return out, res


def kernel(**inputs):
    out, _ = _run(inputs, trace=False)
    return out
